# revision 2
# baseline (speedup 1.0000x reference)
"""Trainium2 Bass kernel for RelationalReasonerV2 (gnn_message_passing).

Strategy: shard node dim N=160 into 8 slices of 20 "i-rows" per core; each
core computes W_t[:, sl, :] and A[:, :, sl, :] for all (b, l).  SPMD: one
program, per-core input data (sliced ne, masks).

On-core layout: H1=128 on partitions, pairs on the free dim.
  h    = relu(pjT + piT[:,i] + const)   -- one fused tensor_scalar per i-row
  MM2  : Wi2 col-tiled at (0,0)/(0,64) so even/odd i-rows land on psum
         partitions 0:64 / 64:128
  relu2: one activation per 2 groups ([128, 960] across 2 psum banks)
  MM3  : block-diag [Wi3;Wi3] stack [128,2] -> logits [2, 480] strips
  evac : engine copy psum->sbuf strips; DMA scatter strips->[rows, j] tiles;
         batched sigmoid (+b3) and mask-mul at the end.
"""

import sys

_RT = "/opt/trn_rl_repo"
if _RT not in sys.path:
    sys.path.insert(0, _RT)

import numpy as np

B, N, D = 4, 160, 64
ZI = ZE = 64
L, LE = 4, 8
H1, H2 = 128, 64
N_CORES = 8
NS = N // N_CORES  # 20 i-rows per core

_cache = {}


def _build():
    import concourse.bacc as bacc
    import concourse.mybir as mybir
    import concourse.tile as tile

    dt = mybir.dt
    f32, bf16 = dt.float32, dt.bfloat16
    Alu = mybir.AluOpType
    Act = mybir.ActivationFunctionType

    nc = bacc.Bacc("TRN2", target_bir_lowering=False, debug=False,
                   num_devices=N_CORES)

    def din(name, shape):
        return nc.dram_tensor(name, shape, f32, kind="ExternalInput").ap()

    neT = din("neT", [B, D, N])
    neTi = din("neTi", [B, D, NS])
    ziT = din("ziT", [ZI, B])
    zeT = din("zeT", [ZE, B])
    lagT = din("lagT", [LE, L])
    Wia = din("Wia", [D, H1]); Wib = din("Wib", [D, H1]); Wiz = din("Wiz", [ZI, H1])
    Wla = din("Wla", [D, H1]); Wlb = din("Wlb", [D, H1]); Wlz = din("Wlz", [ZE, H1])
    Wll = din("Wll", [LE, H1])
    Wi2 = din("Wi2", [H1, H2]); Wl2 = din("Wl2", [H1, H2])
    Wi3s = din("Wi3s", [H1, 2]); Wl3s = din("Wl3s", [H1, 2])
    bi1c = din("bi1c", [H1, 1]); bl1c = din("bl1c", [H1, 1])
    bi2s = din("bi2s", [H1, 1]); bl2s = din("bl2s", [H1, 1])
    bi3c = din("bi3c", [H1, 1]); bl3c = din("bl3c", [H1, 1])
    maskW = din("maskW", [B * NS, N])        # [80, 160]
    maskA = din("maskA", [L * NS, B * N])    # [80, 640] (rows (l,i), col-block b)
    Wsh = nc.dram_tensor("Wsh", [B * NS, N], f32, kind="ExternalOutput").ap()
    Ash = nc.dram_tensor("Ash", [B * L * NS, N], f32, kind="ExternalOutput").ap()

    NPH = B + B * L                       # 20 phases of NS=20 i-rows
    SPP = 1600                            # strip cols per phase (20*160/2)

    with tile.TileContext(nc) as tc:
        with tc.tile_pool(name="const", bufs=1) as cp, \
             tc.tile_pool(name="work", bufs=3) as wp, \
             tc.tile_pool(name="psmm", bufs=2, space="PSUM") as pmm, \
             tc.tile_pool(name="ps3", bufs=2, space="PSUM") as p3:

            # ---------------- stage 0: load inputs ----------------
            neT_s = cp.tile([D, B * N], f32, tag="neT")
            nc.sync.dma_start(
                out=neT_s[:, :].rearrange("p (b n) -> p b n", b=B),
                in_=neT.rearrange("b d n -> d b n"))
            neTi_s = cp.tile([D, B * NS], f32, tag="neTi")
            nc.sync.dma_start(
                out=neTi_s[:, :].rearrange("p (b n) -> p b n", b=B),
                in_=neTi.rearrange("b d n -> d b n"))
            ziT_s = cp.tile([ZI, B], f32, tag="ziT")
            nc.sync.dma_start(out=ziT_s[:, :], in_=ziT)
            zeT_s = cp.tile([ZE, B], f32, tag="zeT")
            nc.sync.dma_start(out=zeT_s[:, :], in_=zeT)
            lagT_s = cp.tile([LE, L], f32, tag="lagT")
            nc.sync.dma_start(out=lagT_s[:, :], in_=lagT)

            w1 = {}
            for nm, ap_, kk in (("ia", Wia, D), ("ib", Wib, D), ("iz", Wiz, ZI),
                                ("la", Wla, D), ("lb", Wlb, D), ("lz", Wlz, ZE),
                                ("ll", Wll, LE)):
                t = cp.tile([kk, H1], f32, tag="w_" + nm)
                nc.sync.dma_start(out=t[:, :], in_=ap_)
                w1[nm] = t

            def load_bf16(name, ap_, p, q):
                tf = cp.tile([p, q], f32, tag=name + "_f")
                nc.sync.dma_start(out=tf[:, :], in_=ap_)
                tb = cp.tile([p, q], bf16, tag=name)
                nc.vector.tensor_copy(tb[:, :], tf[:, :])
                return tb

            w_i2 = load_bf16("w_i2", Wi2, H1, H2)
            w_l2 = load_bf16("w_l2", Wl2, H1, H2)
            w_i3 = load_bf16("w_i3", Wi3s, H1, 2)
            w_l3 = load_bf16("w_l3", Wl3s, H1, 2)

            bias = {}
            for nm, ap_ in (("i1", bi1c), ("l1", bl1c), ("i2", bi2s),
                            ("l2", bl2s), ("i3", bi3c), ("l3", bl3c)):
                t = cp.tile([H1, 1], f32, tag="b_" + nm)
                nc.sync.dma_start(out=t[:, :], in_=ap_)
                bias[nm] = t

            maskW_s = cp.tile([B * NS, N], f32, tag="maskW")
            nc.sync.dma_start(out=maskW_s[:, :], in_=maskW)
            maskA_s = cp.tile([L * NS, B * N], f32, tag="maskA")
            nc.sync.dma_start(out=maskA_s[:, :], in_=maskA)

            # ---------------- stage 1: projections ----------------
            pjT_s = cp.tile([H1, B * N], bf16, tag="pjT")    # + czi folded in
            qjT_s = cp.tile([H1, B * N], bf16, tag="qjT")
            piT_s = cp.tile([H1, B * NS], f32, tag="piT")
            qiTl_s = cp.tile([H1, B * L * NS], f32, tag="qiTl")  # + czl folded
            czi_s = cp.tile([H1, B], f32, tag="czi")
            czl_s = cp.tile([H1, B * L], f32, tag="czl")

            # z / lag projections (shared across b)
            zp = pmm.tile([H1, 1024], f32, tag="mm")
            nc.tensor.matmul(zp[:, 992:996], lhsT=w1["iz"][:, :], rhs=ziT_s[:, :])
            nc.tensor.matmul(zp[:, 996:1000], lhsT=w1["lz"][:, :], rhs=zeT_s[:, :])
            nc.tensor.matmul(zp[:, 1000:1004], lhsT=w1["ll"][:, :], rhs=lagT_s[:, :])
            nc.vector.tensor_scalar(out=czi_s[:, :], in0=zp[:, 992:996],
                                    scalar1=bias["i1"][:, 0:1], scalar2=None,
                                    op0=Alu.add)
            for b in range(B):
                nc.vector.tensor_scalar(out=czl_s[:, b * L:(b + 1) * L],
                                        in0=zp[:, 1000:1004],
                                        scalar1=zp[:, 996 + b:997 + b],
                                        scalar2=bias["l1"][:, 0:1],
                                        op0=Alu.add, op1=Alu.add)

            for b in range(B):
                pp = pmm.tile([H1, 1024], f32, tag="mm")
                nei = neTi_s[:, b * NS:(b + 1) * NS]
                nef = neT_s[:, b * N:(b + 1) * N]
                nc.tensor.matmul(pp[:, 0:NS], lhsT=w1["ia"][:, :], rhs=nei)
                nc.tensor.matmul(pp[:, 32:32 + N], lhsT=w1["ib"][:, :], rhs=nef)
                nc.tensor.matmul(pp[:, 192:192 + NS], lhsT=w1["la"][:, :], rhs=nei)
                nc.tensor.matmul(pp[:, 224:224 + N], lhsT=w1["lb"][:, :], rhs=nef)
                # pjT = pj + czi_b (bf16); qjT = qj (bf16)
                nc.vector.tensor_scalar(out=pjT_s[:, b * N:(b + 1) * N],
                                        in0=pp[:, 32:32 + N],
                                        scalar1=czi_s[:, b:b + 1], scalar2=None,
                                        op0=Alu.add)
                nc.vector.tensor_copy(qjT_s[:, b * N:(b + 1) * N],
                                      pp[:, 224:224 + N])
                nc.vector.tensor_copy(piT_s[:, b * NS:(b + 1) * NS], pp[:, 0:NS])
                for l in range(L):
                    c0 = (b * L + l) * NS
                    nc.vector.tensor_scalar(out=qiTl_s[:, c0:c0 + NS],
                                            in0=pp[:, 192:192 + NS],
                                            scalar1=czl_s[:, b * L + l:b * L + l + 1],
                                            scalar2=None, op0=Alu.add)

            # ---------------- stage 2: pair MLP phases ----------------
            strips = cp.tile([2, NPH * SPP], f32, tag="strips")
            stageW = cp.tile([B * NS, N], f32, tag="stageW")
            stageA = cp.tile([L * NS, B * N], f32, tag="stageA")

            phases = [("i", b, 0) for b in range(B)] + \
                     [("l", b, l) for b in range(B) for l in range(L)]

            # groups within a phase: (row_start, rows_per_half)
            GROUPS = [(0, 3), (6, 3), (12, 3), (18, 1)]

            for pidx, (kind, b, l) in enumerate(phases):
                if kind == "i":
                    pj = pjT_s[:, b * N:(b + 1) * N]
                    sc = piT_s[:, b * NS:(b + 1) * NS]
                    w2, w3 = w_i2, w_i3
                    b2 = bias["i2"]
                else:
                    pj = qjT_s[:, b * N:(b + 1) * N]
                    sc = qiTl_s[:, (b * L + l) * NS:(b * L + l + 1) * NS]
                    w2, w3 = w_l2, w_l3
                    b2 = bias["l2"]

                sb = pidx * SPP
                ps2 = pmm.tile([H1, 1024], f32, tag="mm")   # banks: [0:512),[512:1024)
                ps3t = p3.tile([2, 1024], f32, tag="p3")

                g2ab = wp.tile([H1, 960], bf16, tag="g2ab")
                g2c = wp.tile([H1, 480], bf16, tag="g2c")
                g2d = wp.tile([H1, 160], bf16, tag="g2d")

                h_tiles = []
                hb_n = 0
                for gi, (rs, rh) in enumerate(GROUPS):
                    cw = rh * N                       # 480 or 160
                    h_e = wp.tile([H1, 480], bf16, tag="he")
                    h_o = wp.tile([H1, 480], bf16, tag="ho")
                    for q in range(rh):
                        for half, ht in ((0, h_e), (1, h_o)):
                            r = rs + half * rh + q
                            # engine split for h-build: 12 DVE / 8 GPSIMD
                            eng = nc.vector if (hb_n % 5) < 3 else nc.gpsimd
                            hb_n += 1
                            eng.tensor_scalar(out=ht[:, q * N:(q + 1) * N],
                                              in0=pj,
                                              scalar1=sc[:, r:r + 1],
                                              scalar2=0.0,
                                              op0=Alu.add, op1=Alu.max)
                    h_tiles.append((h_e, h_o, cw))

                # MM2: groups 0,1 -> ps2 banks 0,1 ; groups 2,3 -> second tile
                ps2b = pmm.tile([H1, 1024], f32, tag="mm")
                for gi, (h_e, h_o, cw) in enumerate(h_tiles):
                    pst = ps2 if gi < 2 else ps2b
                    c0 = 512 * (gi % 2)
                    nc.tensor.matmul(pst[0:64, c0:c0 + cw], lhsT=w2[:, :],
                                     rhs=h_e[:, 0:cw], tile_position=(0, 0))
                    nc.tensor.matmul(pst[64:128, c0:c0 + cw], lhsT=w2[:, :],
                                     rhs=h_o[:, 0:cw], tile_position=(0, 64))

                # relu2 (+b2): AB batched across 2 banks on ACT; C, D on ACT
                nc.scalar.activation(
                    out=g2ab[:, :].rearrange("p (a c) -> p a c", a=2),
                    in_=ps2[:, :].rearrange("p (a c) -> p a c", a=2)[:, :, 0:480],
                    func=Act.Relu, bias=b2[:, 0:1])
                nc.scalar.activation(out=g2c[:, :], in_=ps2b[:, 0:480],
                                     func=Act.Relu, bias=b2[:, 0:1])
                nc.scalar.activation(out=g2d[:, :], in_=ps2b[:, 512:672],
                                     func=Act.Relu, bias=b2[:, 0:1])

                # MM3 -> logits strips [2, *]
                nc.tensor.matmul(ps3t[0:2, 0:480], lhsT=w3[:, :], rhs=g2ab[:, 0:480])
                nc.tensor.matmul(ps3t[0:2, 512:992], lhsT=w3[:, :], rhs=g2ab[:, 480:960])
                ps3u = p3.tile([2, 1024], f32, tag="p3")
                nc.tensor.matmul(ps3u[0:2, 0:480], lhsT=w3[:, :], rhs=g2c[:, :])
                nc.tensor.matmul(ps3u[0:2, 512:672], lhsT=w3[:, :], rhs=g2d[:, :])

                # evac psum -> strips
                nc.scalar.activation(
                    out=strips[:, sb:sb + 960].rearrange("p (a c) -> p a c", a=2),
                    in_=ps3t[:, :].rearrange("p (a c) -> p a c", a=2)[:, :, 0:480],
                    func=Act.Copy)
                nc.vector.tensor_copy(strips[:, sb + 960:sb + 1440], ps3u[:, 0:480])
                nc.vector.tensor_copy(strips[:, sb + 1440:sb + 1600], ps3u[:, 512:672])

                # scatter strips -> stage tiles (rows = i, cols = j)
                if kind == "i":
                    rbase = b * NS
                    stg = stageW
                    cbase = 0
                else:
                    rbase = l * NS
                    stg = stageA
                    cbase = b * N
                for gi, (rs, rh) in enumerate(GROUPS):
                    scb = sb + gi * 480
                    for half in (0, 1):
                        r0 = rbase + rs + half * rh
                        nc.sync.dma_start(
                            out=stg[r0:r0 + rh, cbase:cbase + N],
                            in_=strips[half:half + 1, scb:scb + rh * N])

            # ---------------- stage 3: sigmoid + mask + store ----------------
            probsW = cp.tile([B * NS, N], f32, tag="probsW")
            probsA = cp.tile([L * NS, B * N], f32, tag="probsA")
            nc.scalar.activation(out=probsW[:, :], in_=stageW[:, :],
                                 func=Act.Sigmoid, bias=bias["i3"][0:B * NS, 0:1])
            nc.vector.tensor_mul(probsW[:, :], probsW[:, :], maskW_s[:, :])
            nc.scalar.activation(out=probsA[:, :], in_=stageA[:, :],
                                 func=Act.Sigmoid, bias=bias["l3"][0:L * NS, 0:1])
            nc.vector.tensor_mul(probsA[:, :], probsA[:, :], maskA_s[:, :])

            nc.sync.dma_start(out=Wsh, in_=probsW[:, :])
            for b in range(B):
                nc.sync.dma_start(out=Ash[b * (L * NS):(b + 1) * (L * NS), :],
                                  in_=probsA[:, b * N:(b + 1) * N])

    nc.compile()
    return nc


def _prepare_in_maps(inputs):
    f = np.float32
    ne = np.asarray(inputs["node_embeddings"], f)
    zi = np.asarray(inputs["z_intra_t"], f)
    ze = np.asarray(inputs["z_inter_t"], f)
    lag = np.asarray(inputs["lag_emb"], f)
    Wi1 = np.asarray(inputs["Wi1"], f); bi1 = np.asarray(inputs["bi1"], f)
    Wi2 = np.asarray(inputs["Wi2"], f); bi2 = np.asarray(inputs["bi2"], f)
    Wi3 = np.asarray(inputs["Wi3"], f); bi3 = np.asarray(inputs["bi3"], f)
    Wl1 = np.asarray(inputs["Wl1"], f); bl1 = np.asarray(inputs["bl1"], f)
    Wl2 = np.asarray(inputs["Wl2"], f); bl2 = np.asarray(inputs["bl2"], f)
    Wl3 = np.asarray(inputs["Wl3"], f); bl3 = np.asarray(inputs["bl3"], f)

    neT = np.ascontiguousarray(ne.transpose(0, 2, 1))         # [B, D, N]
    Wi3s = np.zeros((H1, 2), f); Wi3s[:H2, 0] = Wi3[:, 0]; Wi3s[H2:, 1] = Wi3[:, 0]
    Wl3s = np.zeros((H1, 2), f); Wl3s[:H2, 0] = Wl3[:, 0]; Wl3s[H2:, 1] = Wl3[:, 0]

    common = {
        "neT": neT,
        "ziT": np.ascontiguousarray(zi.T),
        "zeT": np.ascontiguousarray(ze.T),
        "lagT": np.ascontiguousarray(lag.T),
        "Wia": Wi1[:D], "Wib": Wi1[D:2 * D], "Wiz": Wi1[2 * D:],
        "Wla": Wl1[:D], "Wlb": Wl1[D:2 * D],
        "Wlz": Wl1[2 * D:2 * D + ZE], "Wll": Wl1[2 * D + ZE:],
        "Wi2": Wi2, "Wl2": Wl2, "Wi3s": Wi3s, "Wl3s": Wl3s,
        "bi1c": bi1[:, None], "bl1c": bl1[:, None],
        "bi2s": np.concatenate([bi2, bi2])[:, None],
        "bl2s": np.concatenate([bl2, bl2])[:, None],
        "bi3c": np.full((H1, 1), bi3[0], f),
        "bl3c": np.full((H1, 1), bl3[0], f),
    }
    common = {k: np.ascontiguousarray(v) for k, v in common.items()}

    eye = np.eye(N, dtype=f)
    in_maps = []
    for k in range(N_CORES):
        sl = slice(k * NS, (k + 1) * NS)
        m = dict(common)
        m["neTi"] = np.ascontiguousarray(ne[:, sl, :].transpose(0, 2, 1))
        rowblock = 1.0 - eye[sl, :]                      # [NS, N]
        m["maskW"] = np.ascontiguousarray(np.tile(rowblock, (B, 1)))
        m["maskA"] = np.ascontiguousarray(np.tile(np.tile(rowblock, (L, 1)), (1, B)))
        in_maps.append(m)
    return in_maps


def kernel(**inputs):
    from concourse import bass_utils

    if "nc" not in _cache:
        _cache["nc"] = _build()
    nc = _cache["nc"]

    in_maps = _prepare_in_maps(inputs)
    res = bass_utils.run_bass_kernel_spmd(nc, in_maps,
                                          core_ids=list(range(N_CORES)))
    W = np.empty((B, N, N), np.float32)
    A = np.empty((B, L, N, N), np.float32)
    for k in range(N_CORES):
        sl = slice(k * NS, (k + 1) * NS)
        W[:, sl, :] = res.results[k]["Wsh"].reshape(B, NS, N)
        A[:, :, sl, :] = res.results[k]["Ash"].reshape(B, L, NS, N)
    return W, A


# revision 3
# speedup vs baseline: 2.8097x; 2.8097x over previous
"""Trainium2 Bass kernel for RelationalReasonerV2 (gnn_message_passing).

Strategy: shard node dim N=160 into 8 slices of 20 "i-rows" per core; each
core computes W_t[:, sl, :] and A[:, :, sl, :] for all (b, l).  SPMD: one
program, per-core input data (sliced ne, masks).

On-core layout: H1=128 on partitions, pairs on the free dim.
  h    = relu(pjT + piT[:,i] + const)   -- one fused tensor_scalar per i-row
  MM2  : Wi2 col-tiled at (0,0)/(0,64) so even/odd i-rows land on psum
         partitions 0:64 / 64:128
  relu2: one activation per 2 groups ([128, 960] across 2 psum banks)
  MM3  : block-diag [Wi3;Wi3] stack [128,2] -> logits [2, 480] strips
  evac : engine copy psum->sbuf strips; DMA scatter strips->[rows, j] tiles;
         batched sigmoid (+b3) and mask-mul at the end.
"""

import sys

_RT = "/opt/trn_rl_repo"
if _RT not in sys.path:
    sys.path.insert(0, _RT)

import numpy as np

B, N, D = 4, 160, 64
ZI = ZE = 64
L, LE = 4, 8
H1, H2 = 128, 64
N_CORES = 8
NS = N // N_CORES  # 20 i-rows per core

_cache = {}


def _build():
    import concourse.bacc as bacc
    import concourse.mybir as mybir
    import concourse.tile as tile

    dt = mybir.dt
    f32, bf16 = dt.float32, dt.bfloat16
    Alu = mybir.AluOpType
    Act = mybir.ActivationFunctionType

    nc = bacc.Bacc("TRN2", target_bir_lowering=False, debug=False,
                   num_devices=N_CORES)

    def din(name, shape):
        return nc.dram_tensor(name, shape, f32, kind="ExternalInput").ap()

    neT = din("neT", [B, D, N])
    neTi = din("neTi", [B, D, NS])
    ziT = din("ziT", [ZI, B])
    zeT = din("zeT", [ZE, B])
    lagT = din("lagT", [LE, L])
    Wia = din("Wia", [D, H1]); Wib = din("Wib", [D, H1]); Wiz = din("Wiz", [ZI, H1])
    Wla = din("Wla", [D, H1]); Wlb = din("Wlb", [D, H1]); Wlz = din("Wlz", [ZE, H1])
    Wll = din("Wll", [LE, H1])
    Wi2 = din("Wi2", [H1, H2]); Wl2 = din("Wl2", [H1, H2])
    Wi3s = din("Wi3s", [H1, 2]); Wl3s = din("Wl3s", [H1, 2])
    bi1c = din("bi1c", [H1, 1]); bl1c = din("bl1c", [H1, 1])
    bi2s = din("bi2s", [H1, 1]); bl2s = din("bl2s", [H1, 1])
    bi3c = din("bi3c", [H1, 1]); bl3c = din("bl3c", [H1, 1])
    maskW = din("maskW", [B * NS, N])        # [80, 160]
    maskA = din("maskA", [L * NS, B * N])    # [80, 640] (rows (l,i), col-block b)
    Wsh = nc.dram_tensor("Wsh", [B * NS, N], f32, kind="ExternalOutput").ap()
    Ash = nc.dram_tensor("Ash", [B * L * NS, N], f32, kind="ExternalOutput").ap()

    NPH = B + B * L                       # 20 phases of NS=20 i-rows
    SPP = 1600                            # strip cols per phase (20*160/2)

    with tile.TileContext(nc) as tc:
        with tc.tile_pool(name="const", bufs=1) as cp, \
             tc.tile_pool(name="work", bufs=3) as wp, \
             tc.tile_pool(name="psmm", bufs=2, space="PSUM") as pmm, \
             tc.tile_pool(name="ps3", bufs=2, space="PSUM") as p3:

            # ---------------- stage 0: load inputs ----------------
            neT_s = cp.tile([D, B * N], f32, tag="neT")
            nc.sync.dma_start(
                out=neT_s[:, :].rearrange("p (b n) -> p b n", b=B),
                in_=neT.rearrange("b d n -> d b n"))
            neTi_s = cp.tile([D, B * NS], f32, tag="neTi")
            nc.sync.dma_start(
                out=neTi_s[:, :].rearrange("p (b n) -> p b n", b=B),
                in_=neTi.rearrange("b d n -> d b n"))
            ziT_s = cp.tile([ZI, B], f32, tag="ziT")
            nc.sync.dma_start(out=ziT_s[:, :], in_=ziT)
            zeT_s = cp.tile([ZE, B], f32, tag="zeT")
            nc.sync.dma_start(out=zeT_s[:, :], in_=zeT)
            lagT_s = cp.tile([LE, L], f32, tag="lagT")
            nc.sync.dma_start(out=lagT_s[:, :], in_=lagT)

            w1 = {}
            for nm, ap_, kk in (("ia", Wia, D), ("ib", Wib, D), ("iz", Wiz, ZI),
                                ("la", Wla, D), ("lb", Wlb, D), ("lz", Wlz, ZE),
                                ("ll", Wll, LE)):
                t = cp.tile([kk, H1], f32, tag="w_" + nm)
                nc.sync.dma_start(out=t[:, :], in_=ap_)
                w1[nm] = t

            def load_bf16(name, ap_, p, q):
                tf = cp.tile([p, q], f32, tag=name + "_f")
                nc.sync.dma_start(out=tf[:, :], in_=ap_)
                tb = cp.tile([p, q], bf16, tag=name)
                nc.vector.tensor_copy(tb[:, :], tf[:, :])
                return tb

            w_i2 = load_bf16("w_i2", Wi2, H1, H2)
            w_l2 = load_bf16("w_l2", Wl2, H1, H2)
            w_i3 = load_bf16("w_i3", Wi3s, H1, 2)
            w_l3 = load_bf16("w_l3", Wl3s, H1, 2)

            bias = {}
            for nm, ap_ in (("i1", bi1c), ("l1", bl1c), ("i2", bi2s),
                            ("l2", bl2s), ("i3", bi3c), ("l3", bl3c)):
                t = cp.tile([H1, 1], f32, tag="b_" + nm)
                nc.sync.dma_start(out=t[:, :], in_=ap_)
                bias[nm] = t

            maskW_s = cp.tile([B * NS, N], f32, tag="maskW")
            nc.sync.dma_start(out=maskW_s[:, :], in_=maskW)
            maskA_s = cp.tile([L * NS, B * N], f32, tag="maskA")
            nc.sync.dma_start(out=maskA_s[:, :], in_=maskA)

            # ---------------- stage 1: projections ----------------
            pjT_s = cp.tile([H1, B * N], bf16, tag="pjT")    # + czi folded in
            qjT_s = cp.tile([H1, B * N], bf16, tag="qjT")
            piT_s = cp.tile([H1, B * NS], f32, tag="piT")
            qiTl_s = cp.tile([H1, B * L * NS], f32, tag="qiTl")  # + czl folded
            czi_s = cp.tile([H1, B], f32, tag="czi")
            czl_s = cp.tile([H1, B * L], f32, tag="czl")

            # z / lag projections (shared across b)
            zp = pmm.tile([H1, 1024], f32, tag="mm")
            nc.tensor.matmul(zp[:, 992:996], lhsT=w1["iz"][:, :], rhs=ziT_s[:, :])
            nc.tensor.matmul(zp[:, 996:1000], lhsT=w1["lz"][:, :], rhs=zeT_s[:, :])
            nc.tensor.matmul(zp[:, 1000:1004], lhsT=w1["ll"][:, :], rhs=lagT_s[:, :])
            nc.vector.tensor_scalar(out=czi_s[:, :], in0=zp[:, 992:996],
                                    scalar1=bias["i1"][:, 0:1], scalar2=None,
                                    op0=Alu.add)
            for b in range(B):
                nc.vector.tensor_scalar(out=czl_s[:, b * L:(b + 1) * L],
                                        in0=zp[:, 1000:1004],
                                        scalar1=zp[:, 996 + b:997 + b],
                                        scalar2=bias["l1"][:, 0:1],
                                        op0=Alu.add, op1=Alu.add)

            for b in range(B):
                pp = pmm.tile([H1, 1024], f32, tag="mm")
                nei = neTi_s[:, b * NS:(b + 1) * NS]
                nef = neT_s[:, b * N:(b + 1) * N]
                nc.tensor.matmul(pp[:, 0:NS], lhsT=w1["ia"][:, :], rhs=nei)
                nc.tensor.matmul(pp[:, 32:32 + N], lhsT=w1["ib"][:, :], rhs=nef)
                nc.tensor.matmul(pp[:, 192:192 + NS], lhsT=w1["la"][:, :], rhs=nei)
                nc.tensor.matmul(pp[:, 224:224 + N], lhsT=w1["lb"][:, :], rhs=nef)
                # pjT = pj + czi_b (bf16); qjT = qj (bf16)
                nc.vector.tensor_scalar(out=pjT_s[:, b * N:(b + 1) * N],
                                        in0=pp[:, 32:32 + N],
                                        scalar1=czi_s[:, b:b + 1], scalar2=None,
                                        op0=Alu.add)
                nc.vector.tensor_copy(qjT_s[:, b * N:(b + 1) * N],
                                      pp[:, 224:224 + N])
                nc.vector.tensor_copy(piT_s[:, b * NS:(b + 1) * NS], pp[:, 0:NS])
                for l in range(L):
                    c0 = (b * L + l) * NS
                    nc.vector.tensor_scalar(out=qiTl_s[:, c0:c0 + NS],
                                            in0=pp[:, 192:192 + NS],
                                            scalar1=czl_s[:, b * L + l:b * L + l + 1],
                                            scalar2=None, op0=Alu.add)

            # ---------------- stage 2: pair MLP phases ----------------
            strips = cp.tile([2, NPH * SPP], f32, tag="strips")
            stageW = cp.tile([B * NS, N], f32, tag="stageW")
            stageA = cp.tile([L * NS, B * N], f32, tag="stageA")

            phases = [("i", b, 0) for b in range(B)] + \
                     [("l", b, l) for b in range(B) for l in range(L)]

            # groups within a phase: (row_start, rows_per_half)
            GROUPS = [(0, 3), (6, 3), (12, 3), (18, 1)]

            for pidx, (kind, b, l) in enumerate(phases):
                if kind == "i":
                    pj = pjT_s[:, b * N:(b + 1) * N]
                    sc = piT_s[:, b * NS:(b + 1) * NS]
                    w2, w3 = w_i2, w_i3
                    b2 = bias["i2"]
                else:
                    pj = qjT_s[:, b * N:(b + 1) * N]
                    sc = qiTl_s[:, (b * L + l) * NS:(b * L + l + 1) * NS]
                    w2, w3 = w_l2, w_l3
                    b2 = bias["l2"]

                sb = pidx * SPP
                ps2 = pmm.tile([H1, 1024], f32, tag="mm")   # banks: [0:512),[512:1024)
                ps3t = p3.tile([2, 1024], f32, tag="p3")

                g2ab = wp.tile([H1, 960], bf16, tag="g2ab")
                g2c = wp.tile([H1, 480], bf16, tag="g2c")
                g2d = wp.tile([H1, 160], bf16, tag="g2d")

                h_tiles = []
                hb_n = 0
                for gi, (rs, rh) in enumerate(GROUPS):
                    cw = rh * N                       # 480 or 160
                    h_e = wp.tile([H1, 480], bf16, tag="he")
                    h_o = wp.tile([H1, 480], bf16, tag="ho")
                    for q in range(rh):
                        for half, ht in ((0, h_e), (1, h_o)):
                            r = rs + half * rh + q
                            # engine split for h-build: 13 DVE / 7 ACT
                            use_act = (hb_n % 20) in (2, 5, 8, 11, 14, 17, 19)
                            hb_n += 1
                            if use_act:
                                nc.scalar.activation(
                                    out=ht[:, q * N:(q + 1) * N], in_=pj,
                                    func=Act.Relu, bias=sc[:, r:r + 1])
                                continue
                            eng = nc.vector
                            eng.tensor_scalar(out=ht[:, q * N:(q + 1) * N],
                                              in0=pj,
                                              scalar1=sc[:, r:r + 1],
                                              scalar2=0.0,
                                              op0=Alu.add, op1=Alu.max)
                    h_tiles.append((h_e, h_o, cw))

                # MM2: groups 0,1 -> ps2 banks 0,1 ; groups 2,3 -> second tile
                ps2b = pmm.tile([H1, 1024], f32, tag="mm")
                for gi, (h_e, h_o, cw) in enumerate(h_tiles):
                    pst = ps2 if gi < 2 else ps2b
                    c0 = 512 * (gi % 2)
                    nc.tensor.matmul(pst[0:64, c0:c0 + cw], lhsT=w2[:, :],
                                     rhs=h_e[:, 0:cw], tile_position=(0, 0))
                    nc.tensor.matmul(pst[64:128, c0:c0 + cw], lhsT=w2[:, :],
                                     rhs=h_o[:, 0:cw], tile_position=(0, 64))

                # relu2 (+b2): AB batched across 2 banks on ACT; C, D on ACT
                nc.scalar.activation(
                    out=g2ab[:, :].rearrange("p (a c) -> p a c", a=2),
                    in_=ps2[:, :].rearrange("p (a c) -> p a c", a=2)[:, :, 0:480],
                    func=Act.Relu, bias=b2[:, 0:1])
                nc.scalar.activation(out=g2c[:, :], in_=ps2b[:, 0:480],
                                     func=Act.Relu, bias=b2[:, 0:1])
                nc.scalar.activation(out=g2d[:, :], in_=ps2b[:, 512:672],
                                     func=Act.Relu, bias=b2[:, 0:1])

                # MM3 -> logits strips [2, *]
                nc.tensor.matmul(ps3t[0:2, 0:480], lhsT=w3[:, :], rhs=g2ab[:, 0:480])
                nc.tensor.matmul(ps3t[0:2, 512:992], lhsT=w3[:, :], rhs=g2ab[:, 480:960])
                ps3u = p3.tile([2, 1024], f32, tag="p3")
                nc.tensor.matmul(ps3u[0:2, 0:480], lhsT=w3[:, :], rhs=g2c[:, :])
                nc.tensor.matmul(ps3u[0:2, 512:672], lhsT=w3[:, :], rhs=g2d[:, :])

                # evac psum -> strips
                nc.scalar.activation(
                    out=strips[:, sb:sb + 960].rearrange("p (a c) -> p a c", a=2),
                    in_=ps3t[:, :].rearrange("p (a c) -> p a c", a=2)[:, :, 0:480],
                    func=Act.Copy)
                nc.vector.tensor_copy(strips[:, sb + 960:sb + 1440], ps3u[:, 0:480])
                nc.vector.tensor_copy(strips[:, sb + 1440:sb + 1600], ps3u[:, 512:672])

                # scatter strips -> stage tiles (rows = i, cols = j)
                if kind == "i":
                    rbase = b * NS
                    stg = stageW
                    cbase = 0
                else:
                    rbase = l * NS
                    stg = stageA
                    cbase = b * N
                for gi, (rs, rh) in enumerate(GROUPS):
                    scb = sb + gi * 480
                    for half in (0, 1):
                        r0 = rbase + rs + half * rh
                        nc.sync.dma_start(
                            out=stg[r0:r0 + rh, cbase:cbase + N],
                            in_=strips[half:half + 1, scb:scb + rh * N])

            # ---------------- stage 3: sigmoid + mask + store ----------------
            probsW = cp.tile([B * NS, N], f32, tag="probsW")
            probsA = cp.tile([L * NS, B * N], f32, tag="probsA")
            nc.scalar.activation(out=probsW[:, :], in_=stageW[:, :],
                                 func=Act.Sigmoid, bias=bias["i3"][0:B * NS, 0:1])
            nc.vector.tensor_mul(probsW[:, :], probsW[:, :], maskW_s[:, :])
            nc.scalar.activation(out=probsA[:, :], in_=stageA[:, :],
                                 func=Act.Sigmoid, bias=bias["l3"][0:L * NS, 0:1])
            nc.vector.tensor_mul(probsA[:, :], probsA[:, :], maskA_s[:, :])

            nc.sync.dma_start(out=Wsh, in_=probsW[:, :])
            for b in range(B):
                nc.sync.dma_start(out=Ash[b * (L * NS):(b + 1) * (L * NS), :],
                                  in_=probsA[:, b * N:(b + 1) * N])

    nc.compile()
    return nc


def _prepare_in_maps(inputs):
    f = np.float32
    ne = np.asarray(inputs["node_embeddings"], f)
    zi = np.asarray(inputs["z_intra_t"], f)
    ze = np.asarray(inputs["z_inter_t"], f)
    lag = np.asarray(inputs["lag_emb"], f)
    Wi1 = np.asarray(inputs["Wi1"], f); bi1 = np.asarray(inputs["bi1"], f)
    Wi2 = np.asarray(inputs["Wi2"], f); bi2 = np.asarray(inputs["bi2"], f)
    Wi3 = np.asarray(inputs["Wi3"], f); bi3 = np.asarray(inputs["bi3"], f)
    Wl1 = np.asarray(inputs["Wl1"], f); bl1 = np.asarray(inputs["bl1"], f)
    Wl2 = np.asarray(inputs["Wl2"], f); bl2 = np.asarray(inputs["bl2"], f)
    Wl3 = np.asarray(inputs["Wl3"], f); bl3 = np.asarray(inputs["bl3"], f)

    neT = np.ascontiguousarray(ne.transpose(0, 2, 1))         # [B, D, N]
    Wi3s = np.zeros((H1, 2), f); Wi3s[:H2, 0] = Wi3[:, 0]; Wi3s[H2:, 1] = Wi3[:, 0]
    Wl3s = np.zeros((H1, 2), f); Wl3s[:H2, 0] = Wl3[:, 0]; Wl3s[H2:, 1] = Wl3[:, 0]

    common = {
        "neT": neT,
        "ziT": np.ascontiguousarray(zi.T),
        "zeT": np.ascontiguousarray(ze.T),
        "lagT": np.ascontiguousarray(lag.T),
        "Wia": Wi1[:D], "Wib": Wi1[D:2 * D], "Wiz": Wi1[2 * D:],
        "Wla": Wl1[:D], "Wlb": Wl1[D:2 * D],
        "Wlz": Wl1[2 * D:2 * D + ZE], "Wll": Wl1[2 * D + ZE:],
        "Wi2": Wi2, "Wl2": Wl2, "Wi3s": Wi3s, "Wl3s": Wl3s,
        "bi1c": bi1[:, None], "bl1c": bl1[:, None],
        "bi2s": np.concatenate([bi2, bi2])[:, None],
        "bl2s": np.concatenate([bl2, bl2])[:, None],
        "bi3c": np.full((H1, 1), bi3[0], f),
        "bl3c": np.full((H1, 1), bl3[0], f),
    }
    common = {k: np.ascontiguousarray(v) for k, v in common.items()}

    eye = np.eye(N, dtype=f)
    in_maps = []
    for k in range(N_CORES):
        sl = slice(k * NS, (k + 1) * NS)
        m = dict(common)
        m["neTi"] = np.ascontiguousarray(ne[:, sl, :].transpose(0, 2, 1))
        rowblock = 1.0 - eye[sl, :]                      # [NS, N]
        m["maskW"] = np.ascontiguousarray(np.tile(rowblock, (B, 1)))
        m["maskA"] = np.ascontiguousarray(np.tile(np.tile(rowblock, (L, 1)), (1, B)))
        in_maps.append(m)
    return in_maps


def kernel(**inputs):
    from concourse import bass_utils

    if "nc" not in _cache:
        _cache["nc"] = _build()
    nc = _cache["nc"]

    in_maps = _prepare_in_maps(inputs)
    res = bass_utils.run_bass_kernel_spmd(nc, in_maps,
                                          core_ids=list(range(N_CORES)))
    W = np.empty((B, N, N), np.float32)
    A = np.empty((B, L, N, N), np.float32)
    for k in range(N_CORES):
        sl = slice(k * NS, (k + 1) * NS)
        W[:, sl, :] = res.results[k]["Wsh"].reshape(B, NS, N)
        A[:, :, sl, :] = res.results[k]["Ash"].reshape(B, L, NS, N)
    return W, A


# revision 11
# speedup vs baseline: 3.1432x; 1.1187x over previous
"""Trainium2 Bass kernel for RelationalReasonerV2 (gnn_message_passing).

Strategy: shard node dim N=160 into 8 slices of 20 "i-rows" per core; each
core computes W_t[:, sl, :] and A[:, :, sl, :] for all (b, l).  SPMD: one
program, per-core input data (sliced ne, masks).

On-core layout: H1=128 on partitions, pairs on the free dim.
  h    = relu(pjT + piT[:,i] + const)   -- one fused tensor_scalar per i-row
  MM2  : Wi2 col-tiled at (0,0)/(0,64) so even/odd i-rows land on psum
         partitions 0:64 / 64:128
  relu2: one activation per 2 groups ([128, 960] across 2 psum banks)
  MM3  : block-diag [Wi3;Wi3] stack [128,2] -> logits [2, 480] strips
  evac : engine copy psum->sbuf strips; DMA scatter strips->[rows, j] tiles;
         batched sigmoid (+b3) and mask-mul at the end.
"""

import sys

_RT = "/opt/trn_rl_repo"
if _RT not in sys.path:
    sys.path.insert(0, _RT)

import numpy as np

B, N, D = 4, 160, 64
ZI = ZE = 64
L, LE = 4, 8
H1, H2 = 128, 64
N_CORES = 8
NS = N // N_CORES  # 20 i-rows per core

_cache = {}


def _build():
    import concourse.bacc as bacc
    import concourse.mybir as mybir
    import concourse.tile as tile

    dt = mybir.dt
    f32, bf16 = dt.float32, dt.bfloat16
    Alu = mybir.AluOpType
    Act = mybir.ActivationFunctionType

    nc = bacc.Bacc("TRN2", target_bir_lowering=False, debug=False,
                   num_devices=N_CORES)

    def din(name, shape):
        return nc.dram_tensor(name, shape, f32, kind="ExternalInput").ap()

    neT = din("neT", [B, D, N])
    neTi = din("neTi", [B, D, NS])
    ziT = din("ziT", [ZI, B])
    zeT = din("zeT", [ZE, B])
    lagT = din("lagT", [LE, L])
    Wia = din("Wia", [D, H1]); Wib = din("Wib", [D, H1]); Wiz = din("Wiz", [ZI, H1])
    Wla = din("Wla", [D, H1]); Wlb = din("Wlb", [D, H1]); Wlz = din("Wlz", [ZE, H1])
    Wll = din("Wll", [LE, H1])
    Wi2 = din("Wi2", [H1, H2]); Wl2 = din("Wl2", [H1, H2])
    Wi3s = din("Wi3s", [H1, 2]); Wl3s = din("Wl3s", [H1, 2])
    bi1c = din("bi1c", [H1, 1]); bl1c = din("bl1c", [H1, 1])
    bi2s = din("bi2s", [H1, 1]); bl2s = din("bl2s", [H1, 1])
    bi3c = din("bi3c", [H1, 1]); bl3c = din("bl3c", [H1, 1])
    maskW = din("maskW", [B * NS, N])        # [80, 160]
    maskA = din("maskA", [L * NS, B * N])    # [80, 640] (rows (l,i), col-block b)
    Wsh = nc.dram_tensor("Wsh", [B * NS, N], f32, kind="ExternalOutput").ap()
    Ash = nc.dram_tensor("Ash", [B * L * NS, N], f32, kind="ExternalOutput").ap()

    NPH = B + B * L                       # 20 phases of NS=20 i-rows
    SPP = 1600                            # strip cols per phase (20*160/2)

    with tile.TileContext(nc) as tc:
        with tc.tile_pool(name="const", bufs=1) as cp, \
             tc.tile_pool(name="work", bufs=3) as wp, \
             tc.tile_pool(name="psmm", bufs=2, space="PSUM") as pmm, \
             tc.tile_pool(name="ps3", bufs=2, space="PSUM") as p3:

            # ---------------- stage 0: load inputs ----------------
            neT_s = cp.tile([D, B * N], f32, tag="neT")
            nc.sync.dma_start(
                out=neT_s[:, :].rearrange("p (b n) -> p b n", b=B),
                in_=neT.rearrange("b d n -> d b n"))
            neTi_s = cp.tile([D, B * NS], f32, tag="neTi")
            nc.sync.dma_start(
                out=neTi_s[:, :].rearrange("p (b n) -> p b n", b=B),
                in_=neTi.rearrange("b d n -> d b n"))
            ziT_s = cp.tile([ZI, B], f32, tag="ziT")
            nc.sync.dma_start(out=ziT_s[:, :], in_=ziT)
            zeT_s = cp.tile([ZE, B], f32, tag="zeT")
            nc.sync.dma_start(out=zeT_s[:, :], in_=zeT)
            lagT_s = cp.tile([LE, L], f32, tag="lagT")
            nc.sync.dma_start(out=lagT_s[:, :], in_=lagT)

            w1 = {}
            for nm, ap_, kk in (("ia", Wia, D), ("ib", Wib, D), ("iz", Wiz, ZI),
                                ("la", Wla, D), ("lb", Wlb, D), ("lz", Wlz, ZE),
                                ("ll", Wll, LE)):
                t = cp.tile([kk, H1], f32, tag="w_" + nm)
                nc.sync.dma_start(out=t[:, :], in_=ap_)
                w1[nm] = t

            def load_bf16(name, ap_, p, q):
                tf = cp.tile([p, q], f32, tag=name + "_f")
                nc.sync.dma_start(out=tf[:, :], in_=ap_)
                tb = cp.tile([p, q], bf16, tag=name)
                nc.vector.tensor_copy(tb[:, :], tf[:, :])
                return tb

            w_i2 = load_bf16("w_i2", Wi2, H1, H2)
            w_l2 = load_bf16("w_l2", Wl2, H1, H2)
            w_i3 = load_bf16("w_i3", Wi3s, H1, 2)
            w_l3 = load_bf16("w_l3", Wl3s, H1, 2)

            bias = {}
            for nm, ap_ in (("i1", bi1c), ("l1", bl1c), ("i2", bi2s),
                            ("l2", bl2s), ("i3", bi3c), ("l3", bl3c)):
                t = cp.tile([H1, 1], f32, tag="b_" + nm)
                nc.sync.dma_start(out=t[:, :], in_=ap_)
                bias[nm] = t

            maskW_s = cp.tile([B * NS, N], f32, tag="maskW")
            nc.sync.dma_start(out=maskW_s[:, :], in_=maskW)
            maskA_s = cp.tile([L * NS, B * N], f32, tag="maskA")
            nc.sync.dma_start(out=maskA_s[:, :], in_=maskA)

            # ---------------- stage 1: projections ----------------
            pjT_s = cp.tile([H1, B * N], bf16, tag="pjT")    # raw pj
            qjT_s = cp.tile([H1, B * N], bf16, tag="qjT")    # raw qj
            piT_s = cp.tile([H1, B * NS], f32, tag="piT")    # + czi folded in
            qiT_s = cp.tile([H1, B * NS], f32, tag="qiT")    # raw qi
            czi_s = cp.tile([H1, B], f32, tag="czi")
            czl_s = cp.tile([H1, B * L], f32, tag="czl")

            # z / lag projections (shared across b)
            zp = pmm.tile([H1, 1024], f32, tag="mm")
            nc.tensor.matmul(zp[:, 992:996], lhsT=w1["iz"][:, :], rhs=ziT_s[:, :])
            nc.tensor.matmul(zp[:, 996:1000], lhsT=w1["lz"][:, :], rhs=zeT_s[:, :])
            nc.tensor.matmul(zp[:, 1000:1004], lhsT=w1["ll"][:, :], rhs=lagT_s[:, :])
            nc.vector.tensor_scalar(out=czi_s[:, :], in0=zp[:, 992:996],
                                    scalar1=bias["i1"][:, 0:1], scalar2=None,
                                    op0=Alu.add)
            for b in range(B):
                nc.vector.tensor_scalar(out=czl_s[:, b * L:(b + 1) * L],
                                        in0=zp[:, 1000:1004],
                                        scalar1=zp[:, 996 + b:997 + b],
                                        scalar2=bias["l1"][:, 0:1],
                                        op0=Alu.add, op1=Alu.add)

            for b in range(B):
                pp = pmm.tile([H1, 1024], f32, tag="mm")
                nei = neTi_s[:, b * NS:(b + 1) * NS]
                nef = neT_s[:, b * N:(b + 1) * N]
                nc.tensor.matmul(pp[:, 0:NS], lhsT=w1["ia"][:, :], rhs=nei)
                nc.tensor.matmul(pp[:, 32:32 + N], lhsT=w1["ib"][:, :], rhs=nef)
                nc.tensor.matmul(pp[:, 192:192 + NS], lhsT=w1["la"][:, :], rhs=nei)
                nc.tensor.matmul(pp[:, 224:224 + N], lhsT=w1["lb"][:, :], rhs=nef)
                # pjT/qjT raw bf16; piT carries czi
                nc.vector.tensor_copy(pjT_s[:, b * N:(b + 1) * N],
                                      pp[:, 32:32 + N])
                nc.vector.tensor_copy(qjT_s[:, b * N:(b + 1) * N],
                                      pp[:, 224:224 + N])
                nc.vector.tensor_scalar(out=piT_s[:, b * NS:(b + 1) * NS],
                                        in0=pp[:, 0:NS],
                                        scalar1=czi_s[:, b:b + 1], scalar2=None,
                                        op0=Alu.add)
                nc.vector.tensor_copy(qiT_s[:, b * NS:(b + 1) * NS],
                                      pp[:, 192:192 + NS])

            # ---------------- stage 2: pair MLP phases ----------------
            stageW = cp.tile([B * NS, N], f32, tag="stageW")
            stageA = cp.tile([L * NS, B * N], f32, tag="stageA")

            # phase order: per b -> inst, then 4 lag phases (s_base shared)
            phases = []
            for b in range(B):
                phases.append(("i", b, 0))
                for l in range(L):
                    phases.append(("l", b, l))

            # groups: g=0..2 cover even rows {3g..3g+2} + odd rows {10+3g..};
            # g=3 (ragged) covers rows {9, 19}.
            hb_n = 0
            sb_n = 0
            for pidx, (kind, b, l) in enumerate(phases):
                if kind == "i":
                    pj = pjT_s[:, b * N:(b + 1) * N]
                    sc = piT_s[:, b * NS:(b + 1) * NS]
                    w2, w3 = w_i2, w_i3
                    b2 = bias["i2"]
                else:
                    w2, w3 = w_l2, w_l3
                    b2 = bias["l2"]

                if kind == "l" and l == 0:
                    # build s_base_b = qi (+) qj  [128, NS*N] bf16 once per b
                    s_base = wp.tile([H1, NS * N], bf16, tag="sbase")
                    qj = qjT_s[:, b * N:(b + 1) * N]
                    qi = qiT_s[:, b * NS:(b + 1) * NS]
                    for r in range(NS):
                        eng = nc.vector if (sb_n % 4) < 3 else None
                        sb_n += 1
                        if eng is None:
                            nc.scalar.activation(
                                out=s_base[:, r * N:(r + 1) * N], in_=qj,
                                func=Act.Identity, bias=qi[:, r:r + 1])
                        else:
                            eng.tensor_scalar(
                                out=s_base[:, r * N:(r + 1) * N], in0=qj,
                                scalar1=qi[:, r:r + 1], scalar2=None,
                                op0=Alu.add)

                h_tiles = []
                for g in range(4):
                    if g < 3:
                        re0, ro0, rh = 3 * g, 10 + 3 * g, 3
                    else:
                        re0, ro0, rh = 9, 19, 1
                    cw = rh * N
                    ht = wp.tile([H1, 960], bf16, tag="h")
                    if kind == "l":
                        # one fused op per group: relu(s_base + czl)
                        in0 = s_base[:, :].rearrange(
                            "p (r x) -> p r x", x=N)[:, re0:re0 + rh, :]
                        in1 = s_base[:, :].rearrange(
                            "p (r x) -> p r x", x=N)[:, ro0:ro0 + rh, :]
                        cz = czl_s[:, b * L + l:b * L + l + 1]
                        eng = nc.vector if (g % 2 == 0) else nc.scalar
                        if eng is nc.vector:
                            nc.vector.tensor_scalar(
                                out=ht[:, 0:cw], in0=in0, scalar1=cz,
                                scalar2=0.0, op0=Alu.add, op1=Alu.max)
                            nc.vector.tensor_scalar(
                                out=ht[:, 480:480 + cw], in0=in1, scalar1=cz,
                                scalar2=0.0, op0=Alu.add, op1=Alu.max)
                        else:
                            nc.scalar.activation(out=ht[:, 0:cw], in_=in0,
                                                 func=Act.Relu, bias=cz)
                            nc.scalar.activation(out=ht[:, 480:480 + cw],
                                                 in_=in1, func=Act.Relu,
                                                 bias=cz)
                    else:
                        for q in range(rh):
                            for half, r in ((0, re0 + q), (1, ro0 + q)):
                                use_act = (hb_n % 3) == 2
                                hb_n += 1
                                dst = ht[:, 480 * half + q * N:
                                         480 * half + (q + 1) * N]
                                if use_act:
                                    nc.scalar.activation(
                                        out=dst, in_=pj, func=Act.Relu,
                                        bias=sc[:, r:r + 1])
                                else:
                                    nc.vector.tensor_scalar(
                                        out=dst, in0=pj,
                                        scalar1=sc[:, r:r + 1], scalar2=0.0,
                                        op0=Alu.add, op1=Alu.max)
                    h_tiles.append((ht, cw))

                # MM2: per pair of groups one 2-bank psum tile
                g2s = []
                for gpair in range(2):
                    ps2 = pmm.tile([H1, 1024], f32, tag="mm")
                    for k in range(2):
                        g = gpair * 2 + k
                        ht, cw = h_tiles[g]
                        c0 = 512 * k
                        nc.tensor.matmul(ps2[0:64, c0:c0 + cw], lhsT=w2[:, :],
                                         rhs=ht[:, 0:cw], tile_position=(0, 0))
                        nc.tensor.matmul(ps2[64:128, c0:c0 + cw], lhsT=w2[:, :],
                                         rhs=ht[:, 480:480 + cw],
                                         tile_position=(0, 64))
                    g2 = wp.tile([H1, 960], bf16, tag="g2")
                    if gpair == 0:
                        nc.scalar.activation(
                            out=g2[:, :].rearrange("p (a c) -> p a c", a=2),
                            in_=ps2[:, :].rearrange(
                                "p (a c) -> p a c", a=2)[:, :, 0:480],
                            func=Act.Relu, bias=b2[:, 0:1])
                    else:
                        nc.scalar.activation(out=g2[:, 0:480],
                                             in_=ps2[:, 0:480],
                                             func=Act.Relu, bias=b2[:, 0:1])
                        nc.vector.tensor_scalar(out=g2[:, 480:640],
                                                in0=ps2[:, 512:672],
                                                scalar1=b2[:, 0:1], scalar2=0.0,
                                                op0=Alu.add, op1=Alu.max)
                    g2s.append(g2)

                # MM3 -> logits [2, *] (row0 = even-half rows, row1 = odd)
                ps3t = p3.tile([2, 1024], f32, tag="p3")
                nc.tensor.matmul(ps3t[0:2, 0:480], lhsT=w3[:, :],
                                 rhs=g2s[0][:, 0:480])
                nc.tensor.matmul(ps3t[0:2, 512:992], lhsT=w3[:, :],
                                 rhs=g2s[0][:, 480:960])
                ps3u = p3.tile([2, 1024], f32, tag="p3")
                nc.tensor.matmul(ps3u[0:2, 0:480], lhsT=w3[:, :],
                                 rhs=g2s[1][:, 0:480])
                nc.tensor.matmul(ps3u[0:2, 512:672], lhsT=w3[:, :],
                                 rhs=g2s[1][:, 480:640])

                # evac psum -> strips [2, 1600] (cols g*480 + q*160 + j)
                strips = wp.tile([2, 1600], f32, tag="strips")
                nc.scalar.activation(
                    out=strips[:, 0:960].rearrange("p (a c) -> p a c", a=2),
                    in_=ps3t[:, :].rearrange("p (a c) -> p a c", a=2)[:, :, 0:480],
                    func=Act.Copy)
                nc.vector.tensor_copy(strips[:, 960:1440], ps3u[:, 0:480])
                nc.vector.tensor_copy(strips[:, 1440:1600], ps3u[:, 512:672])

                # scatter strips -> stage tiles (one DMA per half)
                if kind == "i":
                    rbase, stg, cbase = b * NS, stageW, 0
                else:
                    rbase, stg, cbase = l * NS, stageA, b * N
                for half in (0, 1):
                    r0 = rbase + 10 * half
                    nc.sync.dma_start(
                        out=stg[r0:r0 + 10, cbase:cbase + N],
                        in_=strips[half:half + 1, 0:1600])

            # ---------------- stage 3: sigmoid + mask + store ----------------
            probsW = cp.tile([B * NS, N], f32, tag="probsW")
            probsA = cp.tile([L * NS, B * N], f32, tag="probsA")
            nc.scalar.activation(out=probsW[:, :], in_=stageW[:, :],
                                 func=Act.Sigmoid, bias=bias["i3"][0:B * NS, 0:1])
            nc.vector.tensor_mul(probsW[:, :], probsW[:, :], maskW_s[:, :])
            nc.scalar.activation(out=probsA[:, :], in_=stageA[:, :],
                                 func=Act.Sigmoid, bias=bias["l3"][0:L * NS, 0:1])
            nc.vector.tensor_mul(probsA[:, :], probsA[:, :], maskA_s[:, :])

            nc.sync.dma_start(out=Wsh, in_=probsW[:, :])
            for b in range(B):
                nc.sync.dma_start(out=Ash[b * (L * NS):(b + 1) * (L * NS), :],
                                  in_=probsA[:, b * N:(b + 1) * N])

    nc.compile()
    return nc


def _prepare_in_maps(inputs):
    f = np.float32
    ne = np.asarray(inputs["node_embeddings"], f)
    zi = np.asarray(inputs["z_intra_t"], f)
    ze = np.asarray(inputs["z_inter_t"], f)
    lag = np.asarray(inputs["lag_emb"], f)
    Wi1 = np.asarray(inputs["Wi1"], f); bi1 = np.asarray(inputs["bi1"], f)
    Wi2 = np.asarray(inputs["Wi2"], f); bi2 = np.asarray(inputs["bi2"], f)
    Wi3 = np.asarray(inputs["Wi3"], f); bi3 = np.asarray(inputs["bi3"], f)
    Wl1 = np.asarray(inputs["Wl1"], f); bl1 = np.asarray(inputs["bl1"], f)
    Wl2 = np.asarray(inputs["Wl2"], f); bl2 = np.asarray(inputs["bl2"], f)
    Wl3 = np.asarray(inputs["Wl3"], f); bl3 = np.asarray(inputs["bl3"], f)

    neT = np.ascontiguousarray(ne.transpose(0, 2, 1))         # [B, D, N]
    Wi3s = np.zeros((H1, 2), f); Wi3s[:H2, 0] = Wi3[:, 0]; Wi3s[H2:, 1] = Wi3[:, 0]
    Wl3s = np.zeros((H1, 2), f); Wl3s[:H2, 0] = Wl3[:, 0]; Wl3s[H2:, 1] = Wl3[:, 0]

    common = {
        "neT": neT,
        "ziT": np.ascontiguousarray(zi.T),
        "zeT": np.ascontiguousarray(ze.T),
        "lagT": np.ascontiguousarray(lag.T),
        "Wia": Wi1[:D], "Wib": Wi1[D:2 * D], "Wiz": Wi1[2 * D:],
        "Wla": Wl1[:D], "Wlb": Wl1[D:2 * D],
        "Wlz": Wl1[2 * D:2 * D + ZE], "Wll": Wl1[2 * D + ZE:],
        "Wi2": Wi2, "Wl2": Wl2, "Wi3s": Wi3s, "Wl3s": Wl3s,
        "bi1c": bi1[:, None], "bl1c": bl1[:, None],
        "bi2s": np.concatenate([bi2, bi2])[:, None],
        "bl2s": np.concatenate([bl2, bl2])[:, None],
        "bi3c": np.full((H1, 1), bi3[0], f),
        "bl3c": np.full((H1, 1), bl3[0], f),
    }
    common = {k: np.ascontiguousarray(v) for k, v in common.items()}

    eye = np.eye(N, dtype=f)
    in_maps = []
    for k in range(N_CORES):
        sl = slice(k * NS, (k + 1) * NS)
        m = dict(common)
        m["neTi"] = np.ascontiguousarray(ne[:, sl, :].transpose(0, 2, 1))
        rowblock = 1.0 - eye[sl, :]                      # [NS, N]
        m["maskW"] = np.ascontiguousarray(np.tile(rowblock, (B, 1)))
        m["maskA"] = np.ascontiguousarray(np.tile(np.tile(rowblock, (L, 1)), (1, B)))
        in_maps.append(m)
    return in_maps


def kernel(**inputs):
    from concourse import bass_utils

    if "nc" not in _cache:
        _cache["nc"] = _build()
    nc = _cache["nc"]

    in_maps = _prepare_in_maps(inputs)
    res = bass_utils.run_bass_kernel_spmd(nc, in_maps,
                                          core_ids=list(range(N_CORES)))
    W = np.empty((B, N, N), np.float32)
    A = np.empty((B, L, N, N), np.float32)
    for k in range(N_CORES):
        sl = slice(k * NS, (k + 1) * NS)
        W[:, sl, :] = res.results[k]["Wsh"].reshape(B, NS, N)
        A[:, :, sl, :] = res.results[k]["Ash"].reshape(B, L, NS, N)
    return W, A


# revision 13
# speedup vs baseline: 3.8250x; 1.2169x over previous
"""Trainium2 Bass kernel for RelationalReasonerV2 (gnn_message_passing).

Strategy: shard node dim N=160 into 8 slices of 20 "i-rows" per core; each
core computes W_t[:, sl, :] and A[:, :, sl, :] for all (b, l).  SPMD: one
program, per-core input data (sliced ne, masks).

On-core layout: H1=128 on partitions, pairs on the free dim.
  h    = relu(pjT + piT[:,i] + const)   -- one fused tensor_scalar per i-row
  MM2  : Wi2 col-tiled at (0,0)/(0,64) so even/odd i-rows land on psum
         partitions 0:64 / 64:128
  relu2: one activation per 2 groups ([128, 960] across 2 psum banks)
  MM3  : block-diag [Wi3;Wi3] stack [128,2] -> logits [2, 480] strips
  evac : engine copy psum->sbuf strips; DMA scatter strips->[rows, j] tiles;
         batched sigmoid (+b3) and mask-mul at the end.
"""

import sys

_RT = "/opt/trn_rl_repo"
if _RT not in sys.path:
    sys.path.insert(0, _RT)

import numpy as np

B, N, D = 4, 160, 64
ZI = ZE = 64
L, LE = 4, 8
H1, H2 = 128, 64
N_CORES = 8
NS = N // N_CORES  # 20 i-rows per core

_cache = {}


def _build():
    import concourse.bacc as bacc
    import concourse.mybir as mybir
    import concourse.tile as tile

    dt = mybir.dt
    f32, bf16 = dt.float32, dt.bfloat16
    Alu = mybir.AluOpType
    Act = mybir.ActivationFunctionType

    nc = bacc.Bacc("TRN2", target_bir_lowering=False, debug=False,
                   num_devices=N_CORES)

    def din(name, shape):
        return nc.dram_tensor(name, shape, f32, kind="ExternalInput").ap()

    neT = din("neT", [B, D, N])
    neTi = din("neTi", [B, D, NS])
    ziT = din("ziT", [ZI, B])
    zeT = din("zeT", [ZE, B])
    lagT = din("lagT", [LE, L])
    Wia = din("Wia", [D, H1]); Wib = din("Wib", [D, H1]); Wiz = din("Wiz", [ZI, H1])
    Wla = din("Wla", [D, H1]); Wlb = din("Wlb", [D, H1]); Wlz = din("Wlz", [ZE, H1])
    Wll = din("Wll", [LE, H1])
    Wi2z = din("Wi2z", [H1, H1]); Wi2n = din("Wi2n", [H1, H1])
    Wl2z = din("Wl2z", [H1, H1]); Wl2n = din("Wl2n", [H1, H1])
    Wi3s = din("Wi3s", [H1, 2]); Wl3s = din("Wl3s", [H1, 2])
    bi1c = din("bi1c", [H1, 1]); bl1c = din("bl1c", [H1, 1])
    bi2s = din("bi2s", [H1, 1]); bl2s = din("bl2s", [H1, 1])
    bi3c = din("bi3c", [H1, 1]); bl3c = din("bl3c", [H1, 1])
    maskW = din("maskW", [B * NS, N])        # [80, 160]
    maskA = din("maskA", [L * NS, B * N])    # [80, 640] (rows (l,i), col-block b)
    Wsh = nc.dram_tensor("Wsh", [B * NS, N], f32, kind="ExternalOutput").ap()
    Ash = nc.dram_tensor("Ash", [B * L * NS, N], f32, kind="ExternalOutput").ap()

    NPH = B + B * L                       # 20 phases of NS=20 i-rows
    SPP = 1600                            # strip cols per phase (20*160/2)

    with tile.TileContext(nc) as tc:
        with tc.tile_pool(name="const", bufs=1) as cp, \
             tc.tile_pool(name="work", bufs=3) as wp, \
             tc.tile_pool(name="psmm", bufs=2, space="PSUM") as pmm, \
             tc.tile_pool(name="ps3", bufs=2, space="PSUM") as p3:

            # ---------------- stage 0: load inputs ----------------
            neT_s = cp.tile([D, B * N], f32, tag="neT")
            nc.sync.dma_start(
                out=neT_s[:, :].rearrange("p (b n) -> p b n", b=B),
                in_=neT.rearrange("b d n -> d b n"))
            neTi_s = cp.tile([D, B * NS], f32, tag="neTi")
            nc.sync.dma_start(
                out=neTi_s[:, :].rearrange("p (b n) -> p b n", b=B),
                in_=neTi.rearrange("b d n -> d b n"))
            ziT_s = cp.tile([ZI, B], f32, tag="ziT")
            nc.sync.dma_start(out=ziT_s[:, :], in_=ziT)
            zeT_s = cp.tile([ZE, B], f32, tag="zeT")
            nc.sync.dma_start(out=zeT_s[:, :], in_=zeT)
            lagT_s = cp.tile([LE, L], f32, tag="lagT")
            nc.sync.dma_start(out=lagT_s[:, :], in_=lagT)

            w1 = {}
            for nm, ap_, kk in (("ia", Wia, D), ("ib", Wib, D), ("iz", Wiz, ZI),
                                ("la", Wla, D), ("lb", Wlb, D), ("lz", Wlz, ZE),
                                ("ll", Wll, LE)):
                t = cp.tile([kk, H1], f32, tag="w_" + nm)
                nc.sync.dma_start(out=t[:, :], in_=ap_)
                w1[nm] = t

            def load_bf16(name, ap_, p, q):
                tf = cp.tile([p, q], f32, tag=name + "_f")
                nc.sync.dma_start(out=tf[:, :], in_=ap_)
                tb = cp.tile([p, q], bf16, tag=name)
                nc.vector.tensor_copy(tb[:, :], tf[:, :])
                return tb

            w_i2z = load_bf16("w_i2z", Wi2z, H1, H1)
            w_i2n = load_bf16("w_i2n", Wi2n, H1, H1)
            w_l2z = load_bf16("w_l2z", Wl2z, H1, H1)
            w_l2n = load_bf16("w_l2n", Wl2n, H1, H1)
            w_i3 = load_bf16("w_i3", Wi3s, H1, 2)
            w_l3 = load_bf16("w_l3", Wl3s, H1, 2)

            bias = {}
            for nm, ap_ in (("i1", bi1c), ("l1", bl1c), ("i2", bi2s),
                            ("l2", bl2s), ("i3", bi3c), ("l3", bl3c)):
                t = cp.tile([H1, 1], f32, tag="b_" + nm)
                nc.sync.dma_start(out=t[:, :], in_=ap_)
                bias[nm] = t

            maskW_s = cp.tile([B * NS, N], f32, tag="maskW")
            nc.sync.dma_start(out=maskW_s[:, :], in_=maskW)
            maskA_s = cp.tile([L * NS, B * N], f32, tag="maskA")
            nc.sync.dma_start(out=maskA_s[:, :], in_=maskA)

            # ---------------- stage 1: projections ----------------
            pjT_s = cp.tile([H1, B * N], bf16, tag="pjT")    # raw pj
            qjT_s = cp.tile([H1, B * N], bf16, tag="qjT")    # raw qj
            piT_s = cp.tile([H1, B * NS], f32, tag="piT")    # + czi folded in
            qiT_s = cp.tile([H1, B * NS], f32, tag="qiT")    # raw qi
            czi_s = cp.tile([H1, B], f32, tag="czi")
            czl_s = cp.tile([H1, B * L], f32, tag="czl")

            # z / lag projections (shared across b)
            zp = pmm.tile([H1, 1024], f32, tag="mm")
            nc.tensor.matmul(zp[:, 992:996], lhsT=w1["iz"][:, :], rhs=ziT_s[:, :])
            nc.tensor.matmul(zp[:, 996:1000], lhsT=w1["lz"][:, :], rhs=zeT_s[:, :])
            nc.tensor.matmul(zp[:, 1000:1004], lhsT=w1["ll"][:, :], rhs=lagT_s[:, :])
            nc.vector.tensor_scalar(out=czi_s[:, :], in0=zp[:, 992:996],
                                    scalar1=bias["i1"][:, 0:1], scalar2=None,
                                    op0=Alu.add)
            for b in range(B):
                nc.vector.tensor_scalar(out=czl_s[:, b * L:(b + 1) * L],
                                        in0=zp[:, 1000:1004],
                                        scalar1=zp[:, 996 + b:997 + b],
                                        scalar2=bias["l1"][:, 0:1],
                                        op0=Alu.add, op1=Alu.add)

            for b in range(B):
                pp = pmm.tile([H1, 1024], f32, tag="mm")
                nei = neTi_s[:, b * NS:(b + 1) * NS]
                nef = neT_s[:, b * N:(b + 1) * N]
                nc.tensor.matmul(pp[:, 0:NS], lhsT=w1["ia"][:, :], rhs=nei)
                nc.tensor.matmul(pp[:, 32:32 + N], lhsT=w1["ib"][:, :], rhs=nef)
                nc.tensor.matmul(pp[:, 192:192 + NS], lhsT=w1["la"][:, :], rhs=nei)
                nc.tensor.matmul(pp[:, 224:224 + N], lhsT=w1["lb"][:, :], rhs=nef)
                # pjT/qjT raw bf16; piT carries czi
                nc.vector.tensor_copy(pjT_s[:, b * N:(b + 1) * N],
                                      pp[:, 32:32 + N])
                nc.vector.tensor_copy(qjT_s[:, b * N:(b + 1) * N],
                                      pp[:, 224:224 + N])
                nc.vector.tensor_scalar(out=piT_s[:, b * NS:(b + 1) * NS],
                                        in0=pp[:, 0:NS],
                                        scalar1=czi_s[:, b:b + 1], scalar2=None,
                                        op0=Alu.add)
                nc.vector.tensor_copy(qiT_s[:, b * NS:(b + 1) * NS],
                                      pp[:, 192:192 + NS])

            # ---------------- stage 2: pair MLP phases ----------------
            stageW = cp.tile([B * NS, N], f32, tag="stageW")
            stageA = cp.tile([L * NS, B * N], f32, tag="stageA")

            # phase order: per b -> inst, then 4 lag phases (s_base shared)
            phases = []
            for b in range(B):
                phases.append(("i", b, 0))
                for l in range(L):
                    phases.append(("l", b, l))

            # groups: g=0..2 cover even rows {3g..3g+2} + odd rows {10+3g..};
            # g=3 (ragged) covers rows {9, 19}.
            hb_n = 0
            sb_n = 0
            for pidx, (kind, b, l) in enumerate(phases):
                if kind == "i":
                    pj = pjT_s[:, b * N:(b + 1) * N]
                    sc = piT_s[:, b * NS:(b + 1) * NS]
                    w2z, w2n, w3 = w_i2z, w_i2n, w_i3
                    b2 = bias["i2"]
                else:
                    w2z, w2n, w3 = w_l2z, w_l2n, w_l3
                    b2 = bias["l2"]

                if kind == "l" and l == 0:
                    # build s_base_b = qi (+) qj  [128, NS*N] bf16 once per b
                    s_base = wp.tile([H1, NS * N], bf16, tag="sbase")
                    qj = qjT_s[:, b * N:(b + 1) * N]
                    qi = qiT_s[:, b * NS:(b + 1) * NS]
                    for r in range(NS):
                        eng = nc.vector if (sb_n % 4) < 3 else None
                        sb_n += 1
                        if eng is None:
                            nc.scalar.activation(
                                out=s_base[:, r * N:(r + 1) * N], in_=qj,
                                func=Act.Identity, bias=qi[:, r:r + 1])
                        else:
                            eng.tensor_scalar(
                                out=s_base[:, r * N:(r + 1) * N], in0=qj,
                                scalar1=qi[:, r:r + 1], scalar2=None,
                                op0=Alu.add)

                h_tiles = []
                for g in range(4):
                    if g < 3:
                        re0, ro0, rh = 3 * g, 10 + 3 * g, 3
                    else:
                        re0, ro0, rh = 9, 19, 1
                    cw = rh * N
                    ht = wp.tile([H1, 960], bf16, tag="h")
                    if kind == "l":
                        # one fused op per group: relu(s_base + czl)
                        in0 = s_base[:, :].rearrange(
                            "p (r x) -> p r x", x=N)[:, re0:re0 + rh, :]
                        in1 = s_base[:, :].rearrange(
                            "p (r x) -> p r x", x=N)[:, ro0:ro0 + rh, :]
                        cz = czl_s[:, b * L + l:b * L + l + 1]
                        if g < 3:
                            in01 = s_base[:, :].rearrange(
                                "p (r x) -> p r x", x=10 * N)[
                                :, :, re0 * N:re0 * N + cw]
                            nc.vector.tensor_scalar(
                                out=ht[:, :].rearrange(
                                    "p (a c) -> p a c", a=2)[:, :, 0:cw],
                                in0=in01, scalar1=cz,
                                scalar2=0.0, op0=Alu.add, op1=Alu.max)
                        else:
                            nc.vector.tensor_scalar(
                                out=ht[:, 0:cw], in0=in0, scalar1=cz,
                                scalar2=0.0, op0=Alu.add, op1=Alu.max)
                            nc.vector.tensor_scalar(
                                out=ht[:, 480:480 + cw], in0=in1, scalar1=cz,
                                scalar2=0.0, op0=Alu.add, op1=Alu.max)
                    else:
                        for q in range(rh):
                            for half, r in ((0, re0 + q), (1, ro0 + q)):
                                use_act = (hb_n % 3) == 2
                                hb_n += 1
                                dst = ht[:, 480 * half + q * N:
                                         480 * half + (q + 1) * N]
                                if use_act:
                                    nc.scalar.activation(
                                        out=dst, in_=pj, func=Act.Relu,
                                        bias=sc[:, r:r + 1])
                                else:
                                    nc.vector.tensor_scalar(
                                        out=dst, in0=pj,
                                        scalar1=sc[:, r:r + 1], scalar2=0.0,
                                        op0=Alu.add, op1=Alu.max)
                    h_tiles.append((ht, cw))

                # MM2: per pair of groups one 2-bank psum tile
                g2s = []
                for gpair in range(2):
                    ps2 = pmm.tile([H1, 1024], f32, tag="mm")
                    for k in range(2):
                        g = gpair * 2 + k
                        ht, cw = h_tiles[g]
                        c0 = 512 * k
                        nc.tensor.matmul(ps2[:, c0:c0 + cw], lhsT=w2z[:, :],
                                         rhs=ht[:, 0:cw],
                                         start=True, stop=False)
                        nc.tensor.matmul(ps2[:, c0:c0 + cw], lhsT=w2n[:, :],
                                         rhs=ht[:, 480:480 + cw],
                                         start=False, stop=True)
                    g2 = wp.tile([H1, 960], bf16, tag="g2")
                    if gpair == 0:
                        nc.scalar.activation(
                            out=g2[:, :].rearrange("p (a c) -> p a c", a=2),
                            in_=ps2[:, :].rearrange(
                                "p (a c) -> p a c", a=2)[:, :, 0:480],
                            func=Act.Relu, bias=b2[:, 0:1])
                    else:
                        nc.scalar.activation(out=g2[:, 0:480],
                                             in_=ps2[:, 0:480],
                                             func=Act.Relu, bias=b2[:, 0:1])
                        nc.vector.tensor_scalar(out=g2[:, 480:640],
                                                in0=ps2[:, 512:672],
                                                scalar1=b2[:, 0:1], scalar2=0.0,
                                                op0=Alu.add, op1=Alu.max)
                    g2s.append(g2)

                # MM3 -> logits [2, *] (row0 = even-half rows, row1 = odd)
                ps3t = p3.tile([2, 1024], f32, tag="p3")
                nc.tensor.matmul(ps3t[0:2, 0:480], lhsT=w3[:, :],
                                 rhs=g2s[0][:, 0:480])
                nc.tensor.matmul(ps3t[0:2, 512:992], lhsT=w3[:, :],
                                 rhs=g2s[0][:, 480:960])
                ps3u = p3.tile([2, 1024], f32, tag="p3")
                nc.tensor.matmul(ps3u[0:2, 0:480], lhsT=w3[:, :],
                                 rhs=g2s[1][:, 0:480])
                nc.tensor.matmul(ps3u[0:2, 512:672], lhsT=w3[:, :],
                                 rhs=g2s[1][:, 480:640])

                # evac psum -> strips [2, 1600] (cols g*480 + q*160 + j)
                strips = wp.tile([2, 1600], f32, tag="strips")
                nc.scalar.activation(
                    out=strips[:, 0:960].rearrange("p (a c) -> p a c", a=2),
                    in_=ps3t[:, :].rearrange("p (a c) -> p a c", a=2)[:, :, 0:480],
                    func=Act.Copy)
                nc.vector.tensor_copy(strips[:, 960:1440], ps3u[:, 0:480])
                nc.vector.tensor_copy(strips[:, 1440:1600], ps3u[:, 512:672])

                # scatter strips -> stage tiles (one DMA per half)
                if kind == "i":
                    rbase, stg, cbase = b * NS, stageW, 0
                else:
                    rbase, stg, cbase = l * NS, stageA, b * N
                for half in (0, 1):
                    r0 = rbase + 10 * half
                    deng = nc.sync if half == 0 else nc.gpsimd
                    deng.dma_start(
                        out=stg[r0:r0 + 10, cbase:cbase + N],
                        in_=strips[half:half + 1, 0:1600])

            # ---------------- stage 3: sigmoid + mask + store ----------------
            probsW = cp.tile([B * NS, N], f32, tag="probsW")
            probsA = cp.tile([L * NS, B * N], f32, tag="probsA")
            nc.scalar.activation(out=probsW[:, :], in_=stageW[:, :],
                                 func=Act.Sigmoid, bias=bias["i3"][0:B * NS, 0:1])
            nc.vector.tensor_mul(probsW[:, :], probsW[:, :], maskW_s[:, :])
            nc.scalar.activation(out=probsA[:, :], in_=stageA[:, :],
                                 func=Act.Sigmoid, bias=bias["l3"][0:L * NS, 0:1])
            nc.vector.tensor_mul(probsA[:, :], probsA[:, :], maskA_s[:, :])

            nc.sync.dma_start(out=Wsh, in_=probsW[:, :])
            for b in range(B):
                nc.sync.dma_start(out=Ash[b * (L * NS):(b + 1) * (L * NS), :],
                                  in_=probsA[:, b * N:(b + 1) * N])

    nc.compile()
    return nc


def _prepare_in_maps(inputs):
    f = np.float32
    ne = np.asarray(inputs["node_embeddings"], f)
    zi = np.asarray(inputs["z_intra_t"], f)
    ze = np.asarray(inputs["z_inter_t"], f)
    lag = np.asarray(inputs["lag_emb"], f)
    Wi1 = np.asarray(inputs["Wi1"], f); bi1 = np.asarray(inputs["bi1"], f)
    Wi2 = np.asarray(inputs["Wi2"], f); bi2 = np.asarray(inputs["bi2"], f)
    Wi3 = np.asarray(inputs["Wi3"], f); bi3 = np.asarray(inputs["bi3"], f)
    Wl1 = np.asarray(inputs["Wl1"], f); bl1 = np.asarray(inputs["bl1"], f)
    Wl2 = np.asarray(inputs["Wl2"], f); bl2 = np.asarray(inputs["bl2"], f)
    Wl3 = np.asarray(inputs["Wl3"], f); bl3 = np.asarray(inputs["bl3"], f)

    neT = np.ascontiguousarray(ne.transpose(0, 2, 1))         # [B, D, N]
    Wi3s = np.zeros((H1, 2), f); Wi3s[:H2, 0] = Wi3[:, 0]; Wi3s[H2:, 1] = Wi3[:, 0]
    Wl3s = np.zeros((H1, 2), f); Wl3s[:H2, 0] = Wl3[:, 0]; Wl3s[H2:, 1] = Wl3[:, 0]

    common = {
        "neT": neT,
        "ziT": np.ascontiguousarray(zi.T),
        "zeT": np.ascontiguousarray(ze.T),
        "lagT": np.ascontiguousarray(lag.T),
        "Wia": Wi1[:D], "Wib": Wi1[D:2 * D], "Wiz": Wi1[2 * D:],
        "Wla": Wl1[:D], "Wlb": Wl1[D:2 * D],
        "Wlz": Wl1[2 * D:2 * D + ZE], "Wll": Wl1[2 * D + ZE:],
        "Wi2z": np.concatenate([Wi2, np.zeros((H1, H2), f)], axis=1),
        "Wi2n": np.concatenate([np.zeros((H1, H2), f), Wi2], axis=1),
        "Wl2z": np.concatenate([Wl2, np.zeros((H1, H2), f)], axis=1),
        "Wl2n": np.concatenate([np.zeros((H1, H2), f), Wl2], axis=1),
        "Wi3s": Wi3s, "Wl3s": Wl3s,
        "bi1c": bi1[:, None], "bl1c": bl1[:, None],
        "bi2s": np.concatenate([bi2, bi2])[:, None],
        "bl2s": np.concatenate([bl2, bl2])[:, None],
        "bi3c": np.full((H1, 1), bi3[0], f),
        "bl3c": np.full((H1, 1), bl3[0], f),
    }
    common = {k: np.ascontiguousarray(v) for k, v in common.items()}

    eye = np.eye(N, dtype=f)
    in_maps = []
    for k in range(N_CORES):
        sl = slice(k * NS, (k + 1) * NS)
        m = dict(common)
        m["neTi"] = np.ascontiguousarray(ne[:, sl, :].transpose(0, 2, 1))
        rowblock = 1.0 - eye[sl, :]                      # [NS, N]
        m["maskW"] = np.ascontiguousarray(np.tile(rowblock, (B, 1)))
        m["maskA"] = np.ascontiguousarray(np.tile(np.tile(rowblock, (L, 1)), (1, B)))
        in_maps.append(m)
    return in_maps


def kernel(**inputs):
    from concourse import bass_utils

    if "nc" not in _cache:
        _cache["nc"] = _build()
    nc = _cache["nc"]

    in_maps = _prepare_in_maps(inputs)
    res = bass_utils.run_bass_kernel_spmd(nc, in_maps,
                                          core_ids=list(range(N_CORES)))
    W = np.empty((B, N, N), np.float32)
    A = np.empty((B, L, N, N), np.float32)
    for k in range(N_CORES):
        sl = slice(k * NS, (k + 1) * NS)
        W[:, sl, :] = res.results[k]["Wsh"].reshape(B, NS, N)
        A[:, :, sl, :] = res.results[k]["Ash"].reshape(B, L, NS, N)
    return W, A
